# revision 33
# baseline (speedup 1.0000x reference)
"""Trainium2 Bass kernel for nn_FCGFAutoencoder (segment_max -> 3-layer MLP decoder).

Strategy (data-parallel over segments, per sharding hint):
  - batch_ids are sorted, so the host finds the 65 segment boundaries with
    searchsorted and repacks features into a [B, cap, C] array, cast to
    fp16 (rel err ~7e-4 through the decoder, far under the 2e-2 gate),
    padded with -65504 (fp16 max-identity).  Each core gets 8 segments.
  - fp16 halves HBM traffic (32MB/core) AND doubles DVE tensor_tensor
    throughput (2x_1P packed mode), so the max-tree (~85us) tracks the
    DMA stream (~93us at the 360 GB/s per-core DMA-engine roofline).
  - ALL feature DMA triggers are emitted before any compute op enters
    either HWDGE ring's sequencer FIFO (a sequencer executes its FIFO in
    order, so a compute op waiting on DVE would stall every later DMA
    trigger behind it and make the stream DVE-paced).  One whole-segment
    DMA per segment (J=1, fewer DVE ops), segments alternating between
    the SP and Act rings; segment 7 is quarter-split so only ~1/4
    segment of tree work trails the last feature byte.
  - Weight schedule: the small weights/biases (packed into one fp16
    tensor + b3) ride the SP ring right behind segment 0's first chunk;
    W3 (3.1MB, only needed by the decoder tail) rides both rings BEHIND
    the features, split into 6 column-chunk tiles the out-layer
    consumes as they land.  The decoder is never weight-gated.
  - Per segment: tensor_max tree [P, L*C] -> [P, RB*C] -> [P, C] fp16;
    cast to f32, PE-transpose, DVE reduce -> gT column.
  - Decoder runs ONCE over all 8 segments after the last tree (its cost
    is dominated by streaming W2/W3 columns through the PE, so one full
    decode costs the same as a half); b3 is folded in as a rank-1
    ones x b3 matmul on the PE and ACT moves PSUM->SBUF, keeping the
    DVE off the tail's critical path.
"""

import os
import sys
import types

sys.path.insert(0, "/opt/trn_rl_repo")

import numpy as np


def _ensure_axon_hooks():
    """Some images lack antenv.axon_hooks; bass_utils imports it when
    trace=True under axon. Install a shim that lazily wires the real
    ctypes-based NTFF hook from trn_agent_boot if present, else degrades
    to no-trace instead of crashing."""
    try:
        import antenv.axon_hooks  # noqa: F401

        return
    except ImportError:
        pass
    try:
        import antenv
    except ImportError:
        return
    mod = types.ModuleType("antenv.axon_hooks")
    _hook = [None]

    def set_axon_ntff_profile_hook(h):
        _hook[0] = h

    def get_axon_ntff_profile_hook():
        if _hook[0] is None:
            try:
                from trn_agent_boot.trn_boot import _ntff_profile_via_ctypes

                _hook[0] = _ntff_profile_via_ctypes("/opt/axon/libaxon_pjrt.so")
            except Exception:
                return None
        return _hook[0]

    mod.set_axon_ntff_profile_hook = set_axon_ntff_profile_hook
    mod.get_axon_ntff_profile_hook = get_axon_ntff_profile_hook
    sys.modules["antenv.axon_hooks"] = mod
    antenv.axon_hooks = mod

N = 4_194_304
C = 32
B = 64
NUM_POINTS = 1024
NCORES = 8
SPC = B // NCORES  # segments per core
P = 128
NEG = -65504.0  # fp16 lowest: max-identity padding
H1, H2, OUT_D = 256, 512, 3 * NUM_POINTS
K1, K2, NT = H1 // P, H2 // P, OUT_D // 512

LAST_RESULTS = None

_build_cache = {}


def _build(cap):
    if cap in _build_cache:
        return _build_cache[cap]

    import concourse.bacc as bacc
    import concourse.tile as tile
    from concourse import mybir
    from concourse.masks import make_identity
    from contextlib import ExitStack

    L = cap // P  # rows per partition per segment
    LH = L // 2  # rows per half chunk (one per HWDGE ring)
    F = LH * C  # free elems per chunk tile
    LQ4 = L // 4  # rows per quarter chunk (segment 7 tail split)

    f32 = mybir.dt.float32
    f16 = mybir.dt.float16
    AX = mybir.AxisListType.X
    nc = bacc.Bacc("TRN2", target_bir_lowering=False)

    feats = nc.dram_tensor("feats", [SPC * cap, C], f16, kind="ExternalInput")
    # small weights + biases packed host-side into ONE fp16 tensor
    # (each HWDGE trigger costs ~1.7us of ring-FIFO dead time)
    wpk16 = nc.dram_tensor(
        "wpk16", [P, K1 * H2 + H1 + K1 + K2], f16, kind="ExternalInput"
    )
    w3 = nc.dram_tensor("w3", [H2, OUT_D], f16, kind="ExternalInput")
    b3t = nc.dram_tensor("b3t", [1, OUT_D], f16, kind="ExternalInput")
    out = nc.dram_tensor("out", [SPC, OUT_D], f32, kind="ExternalOutput")

    # rows: s*cap + p*L + j*LH + i  ->  [s, j, p, (i c)]
    fview = feats[:].rearrange("(s p j i) c -> s j p (i c)", s=SPC, p=P, j=2)
    # quarter-chunk view of the same rows, for the last segment's tail
    fview4 = feats[:].rearrange("(s p j i) c -> s j p (i c)", s=SPC, p=P, j=4)
    w3view = w3[:].rearrange("(k p) n -> p k n", p=P)

    with ExitStack() as ctx:
        tc = ctx.enter_context(tile.TileContext(nc))
        consts = ctx.enter_context(tc.tile_pool(name="consts", bufs=1))
        fpool = ctx.enter_context(tc.tile_pool(name="feat", bufs=4))
        outp = ctx.enter_context(tc.tile_pool(name="outp", bufs=2))
        redp = ctx.enter_context(tc.tile_pool(name="red", bufs=4))
        ptr = ctx.enter_context(tc.tile_pool(name="ptr", bufs=2, space="PSUM"))
        pmm = ctx.enter_context(tc.tile_pool(name="pmm", bufs=2, space="PSUM"))
        pout = ctx.enter_context(tc.tile_pool(name="pout", bufs=2, space="PSUM"))

        ones = consts.tile([1, SPC], f16)
        nc.gpsimd.memset(ones, 1.0)
        ident = consts.tile([P, P], f32)
        make_identity(nc, ident)

        wpk_sb = consts.tile([P, K1 * H2 + H1 + K1 + K2], f16)
        b3_sb = consts.tile([1, OUT_D], f16)
        b1_sb = wpk_sb[:, K1 * H2 + H1 : K1 * H2 + H1 + K1]
        b2_sb = wpk_sb[:, K1 * H2 + H1 + K1 : K1 * H2 + H1 + K1 + K2]
        w3_sb = [
            consts.tile([P, K2, 512], f16, tag=f"w3c{n}", name=f"w3c{n}")
            for n in range(NT)
        ]

        obs = consts.tile([1, 16], f32)
        gT = consts.tile([C, SPC], f32)

        RB = 8  # row-blocks kept per chunk; small levels are overhead-bound

        def chunk_tree(eng, ft, rj, n0):
            # contiguous tree max over the row axis: pairs (i, c) with
            # (i + n/2, c); in-place halving within ft. Stops at RB
            # blocks (tail levels are fixed-overhead-dominated).
            cur = ft
            n = n0
            while n > 2 * RB:
                if n % 2 == 1:
                    eng.tensor_max(
                        cur[:, 0:C], cur[:, 0:C], cur[:, (n - 1) * C : n * C]
                    )
                    n -= 1
                half = n // 2
                eng.tensor_max(
                    cur[:, 0 : half * C],
                    cur[:, 0 : half * C],
                    cur[:, half * C : n * C],
                )
                n = half
            while n % RB:
                eng.tensor_max(cur[:, 0:C], cur[:, 0:C], cur[:, (n - 1) * C : n * C])
                n -= 1
            eng.tensor_max(
                rj[:, :], cur[:, 0 : (n // 2) * C], cur[:, (n // 2) * C : n * C]
            )

        # ---- Phase 1: every DMA trigger, in ring-FIFO order ----------
        # Each segment's two half-chunks stream CONCURRENTLY, one per
        # HWDGE ring (the engines serve co-active rings 50/50), so
        # segments arrive uniformly every ~11.8us instead of in pairs,
        # and only segment 7's second half (quarter-split) trails.
        fts = []
        for s in range(SPC):
            ftA = fpool.tile([P, F], f16, tag="ft0", name=f"ftA{s}")
            ftB = fpool.tile([P, F], f16, tag="ft1", name=f"ftB{s}")
            nc.sync.dma_start(out=ftA, in_=fview[s, 0])
            if s == SPC - 1:
                # Segment 7's ring-B chunk is quarter-split so only ~a
                # quarter segment of tree work trails the last byte.
                nc.scalar.dma_start(out=ftB[:, 0 : F // 2], in_=fview4[s, 2])
                nc.scalar.dma_start(out=ftB[:, F // 2 : F], in_=fview4[s, 3])
            else:
                nc.scalar.dma_start(out=ftB, in_=fview[s, 1])
            fts.append((ftA, ftB))
            if s == 0:
                # Small weights (3 packed triggers) enter the SP FIFO
                # here -- after segment 0's first half, streaming behind
                # it; each HWDGE trigger costs ~1.7us of ring-FIFO dead
                # time, so they are packed host-side into few tensors.
                nc.sync.dma_start(out=wpk_sb, in_=wpk16[:])
                nc.sync.dma_start(out=b3_sb, in_=b3t[:])
        # W3 column chunks ride both rings behind the features; the
        # decoder consumes them as they land.
        for n in range(NT):
            q = nc.sync if n % 2 == 0 else nc.scalar
            q.dma_start(out=w3_sb[n], in_=w3view[:, :, n * 512 : (n + 1) * 512])

        # ---- PE priming (one-wait rule) ------------------------------
        # PE supports only ONE sync wait per instruction; prime it with
        # throwaway single-wait ops so it has observed the identity
        # (Pool lane) and the SP weight lane before the real matmuls.
        with tc.tile_pool(name="prime", bufs=1, space="PSUM") as primep:
            pp = primep.tile([C, P], f32, tag="prime")
            nc.tensor.transpose(
                out=pp[0:C, 0:P], in_=ident[:, 0:C], identity=ident[:, :]
            )
            # fp16 matmul, both operands from the SP weight lane (b3t
            # is the LAST small-weight DMA in that FIFO, so this covers
            # pk32/wpk16/b3 for the PE).
            pp2 = primep.tile([1, P], f32, tag="prime16")
            nc.tensor.matmul(
                pp2[0:1, 0:C],
                b3_sb[0:1, 0:1],
                b3_sb[0:1, 0:C],
                start=True,
                stop=True,
            )
        # ACT observer over the SP weight lane: decoder relus then
        # carry only their PE wait.  Sits after all Act-ring DMA
        # triggers, stalls nothing.
        nc.scalar.copy(out=obs[0:1, 0:1], in_=b3_sb[0:1, 0:1])

        # ---- Phase 2: reduction trees --------------------------------
        for s in range(SPC):
            ftA, ftB = fts[s]
            rj = redp.tile([P, RB * C], f16, tag="rj")
            chunk_tree(nc.vector, ftA, rj, LH)
            rjB = redp.tile([P, RB * C], f16, tag="rjB")
            if s == SPC - 1:
                chunk_tree(nc.vector, ftB[:, 0 : F // 2], rjB, LQ4)
                nc.vector.tensor_max(rj[:, :], rj[:, :], rjB[:, :])
                qrj = redp.tile([P, RB * C], f16, tag="qrj", name="qrj", bufs=1)
                chunk_tree(nc.vector, ftB[:, F // 2 : F], qrj, LQ4)
                nc.vector.tensor_max(rj[:, :], rj[:, :], qrj[:, :])
            else:
                chunk_tree(nc.vector, ftB, rjB, LH)
                nc.vector.tensor_max(rj[:, :], rj[:, :], rjB[:, :])
            n = RB
            while n > 1:
                half = n // 2
                nc.vector.tensor_max(
                    rj[:, 0 : half * C],
                    rj[:, 0 : half * C],
                    rj[:, half * C : n * C],
                )
                n = half
            rs32 = redp.tile([P, C], f32, tag="rs32")
            nc.vector.tensor_copy(out=rs32[:, :], in_=rj[:, 0:C])
            pt = ptr.tile([C, P], f32, tag="pt")
            nc.tensor.transpose(
                out=pt[:, :], in_=rs32[:, :], identity=ident[:, :]
            )
            nc.vector.reduce_max(out=gT[:, s : s + 1], in_=pt[:, :], axis=AX)

        # ---- Decoder (all 8 segments at once) ------------------------
        # empty segments: reference maps -inf -> 0; padding is -65504,
        # so mask = (g > -60000) in {0,1}; g * mask zeroes empties.
        mask = consts.tile([C, SPC], f32)
        gfix = consts.tile([C, SPC], f32)
        nc.vector.tensor_scalar(
            out=mask[:, :],
            in0=gT[:, :],
            scalar1=-60000.0,
            scalar2=None,
            op0=mybir.AluOpType.is_gt,
        )
        nc.vector.tensor_mul(gfix[:, :], gT[:, :], mask[:, :])
        g16 = consts.tile([C, SPC], f16)
        nc.vector.tensor_copy(out=g16[:, :], in_=gfix[:, :])

        # h1T[m] = relu(W1[:, m]^T @ g + b1[m])   [128, SPC] per chunk m
        h1_sb = consts.tile([P, K1, SPC], f16)
        for m in range(K1):
            pm = pmm.tile([P, SPC], f32, tag="pm")
            nc.tensor.matmul(
                pm[:, :],
                wpk_sb[0:C, K1 * H2 + m * P : K1 * H2 + (m + 1) * P],
                g16[:, :],
                start=True,
                stop=True,
            )
            nc.scalar.activation(
                out=h1_sb[:, m, :],
                in_=pm[:, :],
                func=mybir.ActivationFunctionType.Relu,
                bias=b1_sb[:, m : m + 1],
                scale=1.0,
            )

        # h2T[m] = relu(sum_k W2[k, :, m]^T @ h1T[k] + b2[m])
        h2_sb = consts.tile([P, K2, SPC], f16)
        for m in range(K2):
            pm = pmm.tile([P, SPC], f32, tag="pm")
            for k in range(K1):
                nc.tensor.matmul(
                    pm[:, :],
                    wpk_sb[:, k * H2 + m * P : k * H2 + (m + 1) * P],
                    h1_sb[:, k, :],
                    start=(k == 0),
                    stop=(k == K1 - 1),
                )
            nc.scalar.activation(
                out=h2_sb[:, m, :],
                in_=pm[:, :],
                func=mybir.ActivationFunctionType.Relu,
                bias=b2_sb[:, m : m + 1],
                scale=1.0,
            )

        # out[:, n] = sum_k h2T[k]^T @ W3[k, :, n] + ones^T @ b3[:, n]
        # (b3 enters as a rank-1 matmul so the DVE stays off the tail).
        # The ACT observer copy per W3 chunk folds that chunk's Act-ring
        # DMA completion into ACT's clock, so each matmul group needs
        # only its single ACT wait.
        nc.scalar.copy(out=obs[0:1, 1:2], in_=w3_sb[0][0:1, 0, 0:1])
        for n in range(NT):
            if n + 1 < NT:
                # next chunk's observer BEFORE this chunk's PSUM->SBUF
                # copy: by the time the copies run all W3 chunks have
                # landed, so the observer never blocks the copy chain,
                # and matmul group n+1 needs only its single ACT wait.
                nc.scalar.copy(
                    out=obs[0:1, n + 2 : n + 3], in_=w3_sb[n + 1][0:1, 0, 0:1]
                )
            po = pout.tile([SPC, 512], f32, tag="po")
            for k in range(K2):
                nc.tensor.matmul(
                    po[:, :],
                    h2_sb[:, k, :],
                    w3_sb[n][:, k, :],
                    start=(k == 0),
                    stop=False,
                )
            nc.tensor.matmul(
                po[:, :],
                ones[:, :],
                b3_sb[:, n * 512 : (n + 1) * 512],
                start=False,
                stop=True,
            )
            # alternate the PSUM->SBUF move between ACT and the (idle)
            # DVE so the copy chain isn't serialized on one engine
            if n % 2 == 0:
                ob = outp.tile([SPC, 512], f32, tag="obA", name=f"obA{n}")
                nc.scalar.copy(out=ob[:, :], in_=po[:, :])
            else:
                ob = outp.tile([SPC, 512], f32, tag="obB", name=f"obB{n}")
                nc.vector.tensor_copy(out=ob[:, :], in_=po[:, :])
            # SWDGE store: DMASW lanes unused by the feature stream.
            nc.gpsimd.dma_start(
                out=out[:, n * 512 : (n + 1) * 512],
                in_=ob[:, :],
            )
    nc.compile()
    _build_cache[cap] = nc
    return nc


def kernel(**inputs):
    global LAST_RESULTS
    features = np.asarray(inputs["features"], dtype=np.float32)
    batch_ids = np.asarray(inputs["batch_ids"])
    W1 = np.asarray(inputs["W1"], dtype=np.float32)
    b1 = np.asarray(inputs["b1"], dtype=np.float32)
    W2 = np.asarray(inputs["W2"], dtype=np.float32)
    b2 = np.asarray(inputs["b2"], dtype=np.float32)
    W3 = np.asarray(inputs["W3"], dtype=np.float32)
    b3 = np.asarray(inputs["b3"], dtype=np.float32)

    bounds = np.searchsorted(batch_ids, np.arange(B + 1), side="left")
    seg_len = np.diff(bounds)
    maxlen = max(1, int(seg_len.max()))
    L = -(-maxlen // P)  # ceil
    L = -(-L // 4) * 4  # multiple of 4 (quarter-chunk view of segment 7)
    L = max(L, 64)  # keep L//4 >= 2*RB so the tree structure holds
    cap = L * P

    packed = np.empty((B, cap, C), np.float16)
    for b in range(B):
        lo, hi = int(bounds[b]), int(bounds[b + 1])
        n = hi - lo
        packed[b, :n] = features[lo:hi]
        packed[b, n:] = NEG

    w3h = np.ascontiguousarray(W3.astype(np.float16))
    b3t = np.ascontiguousarray(b3.astype(np.float16).reshape(1, OUT_D))
    # pack the small weights + biases into ONE fp16 tensor:
    # wpk16 = [w2 as p x (k n) | w1 rows | b1t | b2t]
    wpk16 = np.zeros((P, K1 * H2 + H1 + K1 + K2), np.float16)
    wpk16[:, 0 : K1 * H2] = (
        W2.astype(np.float16).reshape(K1, P, H2).transpose(1, 0, 2).reshape(P, K1 * H2)
    )
    wpk16[0:C, K1 * H2 : K1 * H2 + H1] = W1.astype(np.float16)
    wpk16[:, K1 * H2 + H1 : K1 * H2 + H1 + K1] = (
        b1.reshape(K1, P).T.astype(np.float16)
    )
    wpk16[:, K1 * H2 + H1 + K1 :] = b2.reshape(K2, P).T.astype(np.float16)

    nc = _build(cap)

    in_maps = []
    for d in range(NCORES):
        in_maps.append(
            {
                "feats": packed[d * SPC : (d + 1) * SPC].reshape(SPC * cap, C),
                "wpk16": wpk16,
                "w3": w3h,
                "b3t": b3t,
            }
        )

    _ensure_axon_hooks()
    from concourse.bass_utils import run_bass_kernel_spmd

    core_ids = list(range(NCORES))
    try:
        res = run_bass_kernel_spmd(nc, in_maps, core_ids=core_ids)
    except Exception:
        if os.environ.get("BASS_TRACE") and not os.environ.get("BASS_NEVER_TRACE"):
            # trace post-processing can fail in restricted containers;
            # retry without tracing so the numeric result still lands.
            os.environ["BASS_NEVER_TRACE"] = "1"
            try:
                res = run_bass_kernel_spmd(nc, in_maps, core_ids=core_ids)
            finally:
                os.environ.pop("BASS_NEVER_TRACE", None)
        else:
            raise
    LAST_RESULTS = res

    full = np.concatenate([r["out"] for r in res.results], axis=0)
    return full.reshape(B, 3, NUM_POINTS)


# revision 34
# speedup vs baseline: 1.0946x; 1.0946x over previous
"""Trainium2 Bass kernel for nn_FCGFAutoencoder (segment_max -> 3-layer MLP decoder).

Strategy (data-parallel over segments, per sharding hint):
  - batch_ids are sorted, so the host finds the 65 segment boundaries with
    searchsorted and repacks features into a [B, cap, C] array, cast to
    fp16 (rel err ~7e-4 through the decoder, far under the 2e-2 gate),
    padded with -65504 (fp16 max-identity).  Each core gets 8 segments.
  - fp16 halves HBM traffic (32MB/core) AND doubles DVE tensor_tensor
    throughput (2x_1P packed mode), so the max-tree (~85us) tracks the
    DMA stream (~93us at the 360 GB/s per-core DMA-engine roofline).
  - ALL feature DMA triggers are emitted before any compute op enters
    either HWDGE ring's sequencer FIFO (a sequencer executes its FIFO in
    order, so a compute op waiting on DVE would stall every later DMA
    trigger behind it and make the stream DVE-paced).  One whole-segment
    DMA per segment (J=1, fewer DVE ops), segments alternating between
    the SP and Act rings; segment 7 is quarter-split so only ~1/4
    segment of tree work trails the last feature byte.
  - Weight schedule: the small weights/biases (packed into one fp16
    tensor + b3) ride the SP ring right behind segment 0's first chunk;
    W3 (3.1MB, only needed by the decoder tail) rides both rings BEHIND
    the features, split into 6 column-chunk tiles the out-layer
    consumes as they land.  The decoder is never weight-gated.
  - Per segment: tensor_max tree [P, L*C] -> [P, RB*C] -> [P, C] fp16;
    cast to f32, PE-transpose, DVE reduce -> gT column.
  - Decoder runs ONCE over all 8 segments after the last tree (its cost
    is dominated by streaming W2/W3 columns through the PE, so one full
    decode costs the same as a half); b3 is folded in as a rank-1
    ones x b3 matmul on the PE and ACT moves PSUM->SBUF, keeping the
    DVE off the tail's critical path.
"""

import os
import sys
import types

sys.path.insert(0, "/opt/trn_rl_repo")

import numpy as np


def _ensure_axon_hooks():
    """Some images lack antenv.axon_hooks; bass_utils imports it when
    trace=True under axon. Install a shim that lazily wires the real
    ctypes-based NTFF hook from trn_agent_boot if present, else degrades
    to no-trace instead of crashing."""
    try:
        import antenv.axon_hooks  # noqa: F401

        return
    except ImportError:
        pass
    try:
        import antenv
    except ImportError:
        return
    mod = types.ModuleType("antenv.axon_hooks")
    _hook = [None]

    def set_axon_ntff_profile_hook(h):
        _hook[0] = h

    def get_axon_ntff_profile_hook():
        if _hook[0] is None:
            try:
                from trn_agent_boot.trn_boot import _ntff_profile_via_ctypes

                _hook[0] = _ntff_profile_via_ctypes("/opt/axon/libaxon_pjrt.so")
            except Exception:
                return None
        return _hook[0]

    mod.set_axon_ntff_profile_hook = set_axon_ntff_profile_hook
    mod.get_axon_ntff_profile_hook = get_axon_ntff_profile_hook
    sys.modules["antenv.axon_hooks"] = mod
    antenv.axon_hooks = mod

N = 4_194_304
C = 32
B = 64
NUM_POINTS = 1024
NCORES = 8
SPC = B // NCORES  # segments per core
P = 128
NEG = -65504.0  # fp16 lowest: max-identity padding
H1, H2, OUT_D = 256, 512, 3 * NUM_POINTS
K1, K2, NT = H1 // P, H2 // P, OUT_D // 512

LAST_RESULTS = None

_build_cache = {}


def _build(cap):
    if cap in _build_cache:
        return _build_cache[cap]

    import concourse.bacc as bacc
    import concourse.tile as tile
    from concourse import mybir
    from concourse.masks import make_identity
    from contextlib import ExitStack

    L = cap // P  # rows per partition per segment
    LH = L // 2  # rows per half chunk (one per HWDGE ring)
    F = LH * C  # free elems per chunk tile
    LQ4 = L // 4  # rows per quarter chunk (segment 7 tail split)

    f32 = mybir.dt.float32
    f16 = mybir.dt.float16
    AX = mybir.AxisListType.X
    nc = bacc.Bacc("TRN2", target_bir_lowering=False)

    feats = nc.dram_tensor("feats", [SPC * cap, C], f16, kind="ExternalInput")
    # small weights + biases packed host-side into ONE fp16 tensor
    # (each HWDGE trigger costs ~1.7us of ring-FIFO dead time)
    wpk16 = nc.dram_tensor(
        "wpk16", [P, K1 * H2 + H1 + K1 + K2], f16, kind="ExternalInput"
    )
    w3 = nc.dram_tensor("w3", [H2, OUT_D], f16, kind="ExternalInput")
    b3t = nc.dram_tensor("b3t", [1, OUT_D], f16, kind="ExternalInput")
    out = nc.dram_tensor("out", [SPC, OUT_D], f32, kind="ExternalOutput")

    # rows: s*cap + p*L + j*LH + i  ->  [s, j, p, (i c)]
    fview = feats[:].rearrange("(s p j i) c -> s j p (i c)", s=SPC, p=P, j=2)
    # quarter-chunk view of the same rows, for the last segment's tail
    fview4 = feats[:].rearrange("(s p j i) c -> s j p (i c)", s=SPC, p=P, j=4)
    w3view = w3[:].rearrange("(k p) n -> p k n", p=P)

    with ExitStack() as ctx:
        tc = ctx.enter_context(tile.TileContext(nc))
        consts = ctx.enter_context(tc.tile_pool(name="consts", bufs=1))
        fpool = ctx.enter_context(tc.tile_pool(name="feat", bufs=5))
        outp = ctx.enter_context(tc.tile_pool(name="outp", bufs=1))
        redp = ctx.enter_context(tc.tile_pool(name="red", bufs=2))
        ptr = ctx.enter_context(tc.tile_pool(name="ptr", bufs=2, space="PSUM"))
        pmm = ctx.enter_context(tc.tile_pool(name="pmm", bufs=2, space="PSUM"))
        pout = ctx.enter_context(tc.tile_pool(name="pout", bufs=2, space="PSUM"))

        ones = consts.tile([1, SPC], f16)
        nc.gpsimd.memset(ones, 1.0)
        ident = consts.tile([P, P], f32)
        make_identity(nc, ident)

        wpk_sb = consts.tile([P, K1 * H2 + H1 + K1 + K2], f16)
        b3_sb = consts.tile([1, OUT_D], f16)
        b1_sb = wpk_sb[:, K1 * H2 + H1 : K1 * H2 + H1 + K1]
        b2_sb = wpk_sb[:, K1 * H2 + H1 + K1 : K1 * H2 + H1 + K1 + K2]
        w3_sb = [
            consts.tile([P, K2, 512], f16, tag=f"w3c{n}", name=f"w3c{n}")
            for n in range(NT)
        ]

        obs = consts.tile([1, 16], f32)
        gT = consts.tile([C, SPC], f32)

        RB = 8  # row-blocks kept per chunk; small levels are overhead-bound

        def chunk_tree(eng, ft, rj, n0):
            # contiguous tree max over the row axis: pairs (i, c) with
            # (i + n/2, c); in-place halving within ft. Stops at RB
            # blocks (tail levels are fixed-overhead-dominated).
            cur = ft
            n = n0
            while n > 2 * RB:
                if n % 2 == 1:
                    eng.tensor_max(
                        cur[:, 0:C], cur[:, 0:C], cur[:, (n - 1) * C : n * C]
                    )
                    n -= 1
                half = n // 2
                eng.tensor_max(
                    cur[:, 0 : half * C],
                    cur[:, 0 : half * C],
                    cur[:, half * C : n * C],
                )
                n = half
            while n % RB:
                eng.tensor_max(cur[:, 0:C], cur[:, 0:C], cur[:, (n - 1) * C : n * C])
                n -= 1
            eng.tensor_max(
                rj[:, :], cur[:, 0 : (n // 2) * C], cur[:, (n // 2) * C : n * C]
            )

        # ---- Phase 1: every DMA trigger, in ring-FIFO order ----------
        # Each segment's two half-chunks stream CONCURRENTLY, one per
        # HWDGE ring (the engines serve co-active rings 50/50), so
        # segments arrive uniformly every ~11.8us instead of in pairs,
        # and only segment 7's second half (quarter-split) trails.
        fts = []
        for s in range(SPC):
            ftA = fpool.tile([P, F], f16, tag="ft0", name=f"ftA{s}")
            ftB = fpool.tile([P, F], f16, tag="ft1", name=f"ftB{s}")
            nc.sync.dma_start(out=ftA, in_=fview[s, 0])
            if s == SPC - 1:
                # Segment 7's ring-B chunk is quarter-split so only ~a
                # quarter segment of tree work trails the last byte.
                nc.scalar.dma_start(out=ftB[:, 0 : F // 2], in_=fview4[s, 2])
                nc.scalar.dma_start(out=ftB[:, F // 2 : F], in_=fview4[s, 3])
            else:
                nc.scalar.dma_start(out=ftB, in_=fview[s, 1])
            fts.append((ftA, ftB))
            if s == 0:
                # Small weights (3 packed triggers) enter the SP FIFO
                # here -- after segment 0's first half, streaming behind
                # it; each HWDGE trigger costs ~1.7us of ring-FIFO dead
                # time, so they are packed host-side into few tensors.
                nc.sync.dma_start(out=wpk_sb, in_=wpk16[:])
                nc.sync.dma_start(out=b3_sb, in_=b3t[:])
        # W3 column chunks ride both rings behind the features; the
        # decoder consumes them as they land.
        for n in range(NT):
            q = nc.sync if n % 2 == 0 else nc.scalar
            q.dma_start(out=w3_sb[n], in_=w3view[:, :, n * 512 : (n + 1) * 512])

        # ---- PE priming (one-wait rule) ------------------------------
        # PE supports only ONE sync wait per instruction; prime it with
        # throwaway single-wait ops so it has observed the identity
        # (Pool lane) and the SP weight lane before the real matmuls.
        with tc.tile_pool(name="prime", bufs=1, space="PSUM") as primep:
            pp = primep.tile([C, P], f32, tag="prime")
            nc.tensor.transpose(
                out=pp[0:C, 0:P], in_=ident[:, 0:C], identity=ident[:, :]
            )
            # fp16 matmul, both operands from the SP weight lane (b3t
            # is the LAST small-weight DMA in that FIFO, so this covers
            # pk32/wpk16/b3 for the PE).
            pp2 = primep.tile([1, P], f32, tag="prime16")
            nc.tensor.matmul(
                pp2[0:1, 0:C],
                b3_sb[0:1, 0:1],
                b3_sb[0:1, 0:C],
                start=True,
                stop=True,
            )
        # ACT observer over the SP weight lane: decoder relus then
        # carry only their PE wait.  Sits after all Act-ring DMA
        # triggers, stalls nothing.
        nc.scalar.copy(out=obs[0:1, 0:1], in_=b3_sb[0:1, 0:1])

        # ---- Phase 2: reduction trees --------------------------------
        for s in range(SPC):
            ftA, ftB = fts[s]
            rj = redp.tile([P, RB * C], f16, tag="rj")
            chunk_tree(nc.vector, ftA, rj, LH)
            rjB = redp.tile([P, RB * C], f16, tag="rjB")
            if s == SPC - 1:
                chunk_tree(nc.vector, ftB[:, 0 : F // 2], rjB, LQ4)
                nc.vector.tensor_max(rj[:, :], rj[:, :], rjB[:, :])
                qrj = redp.tile([P, RB * C], f16, tag="qrj", name="qrj", bufs=1)
                chunk_tree(nc.vector, ftB[:, F // 2 : F], qrj, LQ4)
                nc.vector.tensor_max(rj[:, :], rj[:, :], qrj[:, :])
            else:
                chunk_tree(nc.vector, ftB, rjB, LH)
                nc.vector.tensor_max(rj[:, :], rj[:, :], rjB[:, :])
            n = RB
            while n > 1:
                half = n // 2
                nc.vector.tensor_max(
                    rj[:, 0 : half * C],
                    rj[:, 0 : half * C],
                    rj[:, half * C : n * C],
                )
                n = half
            rs32 = redp.tile([P, C], f32, tag="rs32")
            nc.vector.tensor_copy(out=rs32[:, :], in_=rj[:, 0:C])
            pt = ptr.tile([C, P], f32, tag="pt")
            nc.tensor.transpose(
                out=pt[:, :], in_=rs32[:, :], identity=ident[:, :]
            )
            nc.vector.reduce_max(out=gT[:, s : s + 1], in_=pt[:, :], axis=AX)

        # ---- Decoder (all 8 segments at once) ------------------------
        # empty segments: reference maps -inf -> 0; padding is -65504,
        # so mask = (g > -60000) in {0,1}; g * mask zeroes empties.
        mask = consts.tile([C, SPC], f32)
        gfix = consts.tile([C, SPC], f32)
        nc.vector.tensor_scalar(
            out=mask[:, :],
            in0=gT[:, :],
            scalar1=-60000.0,
            scalar2=None,
            op0=mybir.AluOpType.is_gt,
        )
        nc.vector.tensor_mul(gfix[:, :], gT[:, :], mask[:, :])
        g16 = consts.tile([C, SPC], f16)
        nc.vector.tensor_copy(out=g16[:, :], in_=gfix[:, :])

        # h1T[m] = relu(W1[:, m]^T @ g + b1[m])   [128, SPC] per chunk m
        h1_sb = consts.tile([P, K1, SPC], f16)
        for m in range(K1):
            pm = pmm.tile([P, SPC], f32, tag="pm")
            nc.tensor.matmul(
                pm[:, :],
                wpk_sb[0:C, K1 * H2 + m * P : K1 * H2 + (m + 1) * P],
                g16[:, :],
                start=True,
                stop=True,
            )
            nc.scalar.activation(
                out=h1_sb[:, m, :],
                in_=pm[:, :],
                func=mybir.ActivationFunctionType.Relu,
                bias=b1_sb[:, m : m + 1],
                scale=1.0,
            )

        # h2T[m] = relu(sum_k W2[k, :, m]^T @ h1T[k] + b2[m])
        h2_sb = consts.tile([P, K2, SPC], f16)
        for m in range(K2):
            pm = pmm.tile([P, SPC], f32, tag="pm")
            for k in range(K1):
                nc.tensor.matmul(
                    pm[:, :],
                    wpk_sb[:, k * H2 + m * P : k * H2 + (m + 1) * P],
                    h1_sb[:, k, :],
                    start=(k == 0),
                    stop=(k == K1 - 1),
                )
            nc.scalar.activation(
                out=h2_sb[:, m, :],
                in_=pm[:, :],
                func=mybir.ActivationFunctionType.Relu,
                bias=b2_sb[:, m : m + 1],
                scale=1.0,
            )

        # out[:, n] = sum_k h2T[k]^T @ W3[k, :, n] + ones^T @ b3[:, n]
        # (b3 enters as a rank-1 matmul so the DVE stays off the tail).
        # The ACT observer copy per W3 chunk folds that chunk's Act-ring
        # DMA completion into ACT's clock, so each matmul group needs
        # only its single ACT wait.
        nc.scalar.copy(out=obs[0:1, 1:2], in_=w3_sb[0][0:1, 0, 0:1])
        for n in range(NT):
            if n + 1 < NT:
                # next chunk's observer BEFORE this chunk's PSUM->SBUF
                # copy: by the time the copies run all W3 chunks have
                # landed, so the observer never blocks the copy chain,
                # and matmul group n+1 needs only its single ACT wait.
                nc.scalar.copy(
                    out=obs[0:1, n + 2 : n + 3], in_=w3_sb[n + 1][0:1, 0, 0:1]
                )
            po = pout.tile([SPC, 512], f32, tag="po")
            for k in range(K2):
                nc.tensor.matmul(
                    po[:, :],
                    h2_sb[:, k, :],
                    w3_sb[n][:, k, :],
                    start=(k == 0),
                    stop=False,
                )
            nc.tensor.matmul(
                po[:, :],
                ones[:, :],
                b3_sb[:, n * 512 : (n + 1) * 512],
                start=False,
                stop=True,
            )
            # alternate the PSUM->SBUF move between ACT and the (idle)
            # DVE so the copy chain isn't serialized on one engine
            if n % 2 == 0:
                ob = outp.tile([SPC, 512], f32, tag="obA", name=f"obA{n}")
                nc.scalar.copy(out=ob[:, :], in_=po[:, :])
            else:
                ob = outp.tile([SPC, 512], f32, tag="obB", name=f"obB{n}")
                nc.vector.tensor_copy(out=ob[:, :], in_=po[:, :])
            # SWDGE store: DMASW lanes unused by the feature stream.
            nc.gpsimd.dma_start(
                out=out[:, n * 512 : (n + 1) * 512],
                in_=ob[:, :],
            )
    nc.compile()
    _build_cache[cap] = nc
    return nc


def kernel(**inputs):
    global LAST_RESULTS
    features = np.asarray(inputs["features"], dtype=np.float32)
    batch_ids = np.asarray(inputs["batch_ids"])
    W1 = np.asarray(inputs["W1"], dtype=np.float32)
    b1 = np.asarray(inputs["b1"], dtype=np.float32)
    W2 = np.asarray(inputs["W2"], dtype=np.float32)
    b2 = np.asarray(inputs["b2"], dtype=np.float32)
    W3 = np.asarray(inputs["W3"], dtype=np.float32)
    b3 = np.asarray(inputs["b3"], dtype=np.float32)

    bounds = np.searchsorted(batch_ids, np.arange(B + 1), side="left")
    seg_len = np.diff(bounds)
    maxlen = max(1, int(seg_len.max()))
    L = -(-maxlen // P)  # ceil
    L = -(-L // 4) * 4  # multiple of 4 (quarter-chunk view of segment 7)
    L = max(L, 64)  # keep L//4 >= 2*RB so the tree structure holds
    cap = L * P

    packed = np.empty((B, cap, C), np.float16)
    for b in range(B):
        lo, hi = int(bounds[b]), int(bounds[b + 1])
        n = hi - lo
        packed[b, :n] = features[lo:hi]
        packed[b, n:] = NEG

    w3h = np.ascontiguousarray(W3.astype(np.float16))
    b3t = np.ascontiguousarray(b3.astype(np.float16).reshape(1, OUT_D))
    # pack the small weights + biases into ONE fp16 tensor:
    # wpk16 = [w2 as p x (k n) | w1 rows | b1t | b2t]
    wpk16 = np.zeros((P, K1 * H2 + H1 + K1 + K2), np.float16)
    wpk16[:, 0 : K1 * H2] = (
        W2.astype(np.float16).reshape(K1, P, H2).transpose(1, 0, 2).reshape(P, K1 * H2)
    )
    wpk16[0:C, K1 * H2 : K1 * H2 + H1] = W1.astype(np.float16)
    wpk16[:, K1 * H2 + H1 : K1 * H2 + H1 + K1] = (
        b1.reshape(K1, P).T.astype(np.float16)
    )
    wpk16[:, K1 * H2 + H1 + K1 :] = b2.reshape(K2, P).T.astype(np.float16)

    nc = _build(cap)

    in_maps = []
    for d in range(NCORES):
        in_maps.append(
            {
                "feats": packed[d * SPC : (d + 1) * SPC].reshape(SPC * cap, C),
                "wpk16": wpk16,
                "w3": w3h,
                "b3t": b3t,
            }
        )

    _ensure_axon_hooks()
    from concourse.bass_utils import run_bass_kernel_spmd

    core_ids = list(range(NCORES))
    try:
        res = run_bass_kernel_spmd(nc, in_maps, core_ids=core_ids)
    except Exception:
        if os.environ.get("BASS_TRACE") and not os.environ.get("BASS_NEVER_TRACE"):
            # trace post-processing can fail in restricted containers;
            # retry without tracing so the numeric result still lands.
            os.environ["BASS_NEVER_TRACE"] = "1"
            try:
                res = run_bass_kernel_spmd(nc, in_maps, core_ids=core_ids)
            finally:
                os.environ.pop("BASS_NEVER_TRACE", None)
        else:
            raise
    LAST_RESULTS = res

    full = np.concatenate([r["out"] for r in res.results], axis=0)
    return full.reshape(B, 3, NUM_POINTS)
